# revision 1
# baseline (speedup 1.0000x reference)
"""Trainium2 Bass kernel for nn_ChimeraV2Block (dual-softmax differential
sliding-window attention block, B=1 S=2048 D=2048, 16 q-heads / 4 kv-heads,
head_dim 128, window 512).

Sharding: tensor-parallel over heads across 8 NeuronCores. Core c owns
q-heads {2c, 2c+1} and kv-head c//2 (GQA groups align with the split).
Wq/Wk/Wv column-sharded, Wo row-sharded; the 8 fp32 partial outputs are
summed on the host (the "all-reduce").
"""

import sys

if "/opt/trn_rl_repo" not in sys.path:
    sys.path.insert(0, "/opt/trn_rl_repo")

import numpy as np
import ml_dtypes

BF = ml_dtypes.bfloat16

S = 2048
D = 2048
H = 16
HK = 4
HD = 128
WIN = 512
THETA = 10000.0
N_CORES = 8
NQT = S // 128          # 16 q row-tiles
NKT = D // 128          # 16 contraction tiles for the projections
WMAX = WIN + 128        # 640: max key-window width per q-tile
NEG = -1.0e30

_CACHE = {}


def _tables():
    """RoPE tables [128, S] fp32 with head-dim-duplicated frequencies
    (row p uses invf[p % 64]), so every op reads the table at the same
    base partition as its (possibly swapped) q operand. Q tables are
    pre-scaled by the attention scale 1/sqrt(64)."""
    invf = 1.0 / (THETA ** (np.arange(0, HD, 2, dtype=np.float64) / HD))  # [64]
    t = np.arange(S, dtype=np.float64)
    fr = np.outer(invf, t)  # [64, S]
    cosf = np.concatenate([np.cos(fr)] * 2, axis=0)
    sinf = np.concatenate([np.sin(fr)] * 2, axis=0)
    return (np.ascontiguousarray(cosf * 0.125, dtype=np.float32),
            np.ascontiguousarray(sinf * 0.125, dtype=np.float32),
            np.ascontiguousarray(cosf, dtype=np.float32),
            np.ascontiguousarray(sinf, dtype=np.float32))


def _masks():
    p = np.arange(128)[:, None]
    c = np.arange(WMAX)[None, :]
    band = (c - p >= 1) & (c - p <= WIN)
    mw = np.where(band, 0.0, NEG).astype(BF)          # [128, 640]
    cc = np.arange(128)[None, :]
    mc = np.where(cc <= p, 0.0, NEG).astype(BF)       # [128, 128] causal
    # edge mask: cols [0,512) allowed, cols [512,640) causal triangle.
    # slicing the last w cols gives the mask for edge q-tiles (qi < 4).
    me = np.zeros((128, WMAX), dtype=BF)
    me[:, WIN:] = mc
    return mw, me


def _build_program():
    import concourse.bacc as bacc
    import concourse.tile as tile
    from concourse import mybir

    bf = mybir.dt.bfloat16
    f32 = mybir.dt.float32
    EXP = mybir.ActivationFunctionType.Exp
    MULT = mybir.AluOpType.mult
    ADD = mybir.AluOpType.add
    MAX = mybir.AluOpType.max
    DIV = mybir.AluOpType.divide

    nc = bacc.Bacc("TRN2", target_bir_lowering=False, debug=False,
                   num_devices=N_CORES)

    xt_d = nc.dram_tensor("xt", [128, NKT, S], bf, kind="ExternalInput")
    wq_d = nc.dram_tensor("wq", [128, NKT, 2, 128], bf, kind="ExternalInput")
    wk_d = nc.dram_tensor("wk", [128, NKT, 128], bf, kind="ExternalInput")
    wv_d = nc.dram_tensor("wv", [128, NKT, 128], bf, kind="ExternalInput")
    wo_d = nc.dram_tensor("wo", [128, 2, D], bf, kind="ExternalInput")
    lamn_d = nc.dram_tensor("lamn", [1, 2], f32, kind="ExternalInput")
    f16 = mybir.dt.float16
    out_d = nc.dram_tensor("outp", [S, D], f16, kind="ExternalOutput")

    tqc_np, tqs_np, tkc_np, tks_np = _tables()
    mw_np, me_np = _masks()
    tqc_d = nc.inline_tensor(tqc_np, "tab_qc")
    tqs_d = nc.inline_tensor(tqs_np, "tab_qs")
    tkc_d = nc.inline_tensor(tkc_np, "tab_kc")
    tks_d = nc.inline_tensor(tks_np, "tab_ks")
    mw_d = nc.inline_tensor(mw_np, "mask_win")
    me_d = nc.inline_tensor(me_np, "mask_edge")
    idb_d = nc.inline_tensor(np.eye(128, dtype=BF), "ident_bf")
    idf_d = nc.inline_tensor(np.eye(128, dtype=np.float32), "ident_f32")

    with tile.TileContext(nc) as tc:
        with tc.tile_pool(name="xpool", bufs=1) as xp, \
             tc.tile_pool(name="wpool", bufs=1) as wp, \
             tc.tile_pool(name="pers", bufs=1) as pers:

            xt = xp.tile([128, NKT, S], bf)
            for kti in range(NKT):
                for hh in range(2):
                    sl = slice(hh * (S // 2), (hh + 1) * (S // 2))
                    nc.sync.dma_start(out=xt[:, kti, sl], in_=xt_d[:, kti, sl])
            wq = wp.tile([128, NKT, 2, 128], bf)
            for i in range(4):
                nc.sync.dma_start(out=wq[:, 4 * i:4 * i + 4], in_=wq_d[:, 4 * i:4 * i + 4])
            wk = wp.tile([128, NKT, 128], bf)
            nc.sync.dma_start(out=wk[:, 0:8], in_=wk_d[:, 0:8])
            nc.sync.dma_start(out=wk[:, 8:16], in_=wk_d[:, 8:16])
            wv = wp.tile([128, NKT, 128], bf)
            nc.sync.dma_start(out=wv[:, 0:8], in_=wv_d[:, 0:8])
            nc.sync.dma_start(out=wv[:, 8:16], in_=wv_d[:, 8:16])
            wo = wp.tile([128, 2, D], bf)
            for i in range(4):
                nc.sync.dma_start(out=wo[:, :, 512 * i:512 * (i + 1)],
                                  in_=wo_d[:, :, 512 * i:512 * (i + 1)])
            tqc = wp.tile([128, S], f32)
            tqs = wp.tile([128, S], f32)
            tkc = wp.tile([128, S], f32)
            tks = wp.tile([128, S], f32)
            for i in range(4):
                sl = slice(512 * i, 512 * (i + 1))
                nc.sync.dma_start(out=tqc[:, sl], in_=tqc_d[:, sl])
                nc.sync.dma_start(out=tqs[:, sl], in_=tqs_d[:, sl])
                nc.sync.dma_start(out=tkc[:, sl], in_=tkc_d[:, sl])
                nc.sync.dma_start(out=tks[:, sl], in_=tks_d[:, sl])
            mw = wp.tile([128, WMAX], bf)
            nc.sync.dma_start(out=mw[:], in_=mw_d[:])
            me = wp.tile([128, WMAX], bf)
            nc.sync.dma_start(out=me[:], in_=me_d[:])
            idb = wp.tile([128, 128], bf)
            nc.sync.dma_start(out=idb[:], in_=idb_d[:])
            lamn = wp.tile([1, 2], f32)
            nc.sync.dma_start(out=lamn[:], in_=lamn_d[:])
            lamb = wp.tile([128, 2], f32)
            nc.gpsimd.partition_broadcast(lamb[:], lamn[:])

            qt = pers.tile([128, 2, S], bf)      # RoPE'd scaled q, hd-major
            kt = pers.tile([128, S], bf)         # RoPE'd k, hd-major
            vsm = pers.tile([128, NQT, 128], bf)  # v, S-major [s, hd]
            att = pers.tile([128, 2, S], bf)     # attention out^T, hd-major

            # ---- Phase 1: projections + RoPE + v transpose ----
            with tc.tile_pool(name="pp", bufs=1, space="PSUM") as pp, \
                 tc.tile_pool(name="pt", bufs=2) as pt:
                for nch in range(4):
                    sl = slice(nch * 512, (nch + 1) * 512)
                    ps_q0 = pp.tile([128, 512], f32, tag="pq0", bufs=2)
                    ps_q1 = pp.tile([128, 512], f32, tag="pq1", bufs=2)
                    ps_k = pp.tile([128, 512], f32, tag="pk", bufs=1)
                    ps_v = pp.tile([128, 512], f32, tag="pv", bufs=1)
                    for kti in range(NKT):
                        st = kti == 0
                        sp = kti == NKT - 1
                        rhs = xt[:, kti, sl]
                        nc.tensor.matmul(ps_q0[:], wq[:, kti, 0, :], rhs, start=st, stop=sp)
                        nc.tensor.matmul(ps_q1[:], wq[:, kti, 1, :], rhs, start=st, stop=sp)
                        nc.tensor.matmul(ps_k[:], wk[:, kti, :], rhs, start=st, stop=sp)
                        nc.tensor.matmul(ps_v[:], wv[:, kti, :], rhs, start=st, stop=sp)
                    for ps, outt, tabc, tabs in (
                            (ps_q0, qt[:, 0, sl], tqc, tqs),
                            (ps_q1, qt[:, 1, sl], tqc, tqs),
                            (ps_k, kt[:, sl], tkc, tks)):
                        f = pt.tile([128, 512], f32, tag="f")
                        m1 = pt.tile([128, 512], f32, tag="m1")
                        m2 = pt.tile([128, 512], f32, tag="m2")
                        nc.vector.tensor_copy(out=f[:], in_=ps[:])
                        # m2 = rotate_half partner * sin
                        nc.vector.tensor_mul(m2[0:64, :], f[64:128, :], tabs[64:128, sl])
                        nc.vector.tensor_mul(m2[64:128, :], f[0:64, :], tabs[0:64, sl])
                        nc.vector.tensor_mul(m1[:], f[:], tabc[:, sl])
                        nc.vector.tensor_sub(outt[0:64, :], m1[0:64, :], m2[0:64, :])
                        nc.vector.tensor_add(outt[64:128, :], m1[64:128, :], m2[64:128, :])
                    vtmp = pt.tile([128, 512], bf, tag="vtmp")
                    nc.vector.tensor_copy(out=vtmp[:], in_=ps_v[:])
                    ps_tv = pp.tile([128, 4, 128], bf, tag="ptv", bufs=2)
                    for j in range(4):
                        nc.tensor.transpose(ps_tv[:, j, :], vtmp[:, 128 * j:128 * (j + 1)], idb[:])
                    nc.vector.tensor_copy(out=vsm[:, 4 * nch:4 * (nch + 1), :], in_=ps_tv[:])

            # ---- Phase 2: attention ----
            with tc.tile_pool(name="psc", bufs=1, space="PSUM") as psc, \
                 tc.tile_pool(name="pse", bufs=1) as pse, \
                 tc.tile_pool(name="psm", bufs=1) as psm:
                for qi in range(NQT):
                    qsl = slice(qi * 128, (qi + 1) * 128)
                    kw = min(qi + 1, 5)
                    w = kw * 128
                    kstart = max(0, qi - 4)
                    kwin = slice(kstart * 128, kstart * 128 + w)

                    ps_av0 = psc.tile([128, 128], f32, tag="av0", bufs=1)
                    ps_av1 = psc.tile([128, 128], f32, tag="av1", bufs=1)
                    ps_av = [ps_av0, ps_av1]

                    for h in range(2):
                        ps_s1 = psc.tile([128, WMAX], f32, tag="s", bufs=2)
                        ps_s2 = psc.tile([128, WMAX], f32, tag="s", bufs=2)
                        for half, ps in ((0, ps_s1), (1, ps_s2)):
                            hp = slice(64 * half, 64 * half + 64)
                            lhs = qt[hp, h, qsl]
                            msk = mw if qi >= 4 else me[:, WMAX - w:WMAX]
                            wa = min(w, 512)
                            nc.tensor.matmul(ps[:, 0:wa], idb[:], msk[:, 0:wa],
                                             start=True, stop=False)
                            nc.tensor.matmul(ps[:, 0:wa], lhs,
                                             kt[hp, kwin][:, 0:wa],
                                             start=False, stop=True)
                            if w > 512:
                                nc.tensor.matmul(ps[:, 512:w], idb[:],
                                                 msk[:, 512:w],
                                                 start=True, stop=False)
                                nc.tensor.matmul(ps[:, 512:w], lhs,
                                                 kt[hp, kwin][:, 512:w],
                                                 start=False, stop=True)

                        e1 = pse.tile([128, WMAX], bf, tag="e1", bufs=2)
                        e2 = pse.tile([128, WMAX], bf, tag="e2", bufs=2)
                        s1 = psm.tile([128, 1], f32, tag="s1", bufs=4)
                        s2 = psm.tile([128, 1], f32, tag="s2", bufs=4)
                        nc.scalar.activation(out=e1[:, 0:w], in_=ps_s1[:, 0:w],
                                             func=EXP, accum_out=s1[:])
                        nc.scalar.activation(out=e2[:, 0:w], in_=ps_s2[:, 0:w],
                                             func=EXP, accum_out=s2[:])

                        # cneg = -(lam * s1 / s2)   (lamn holds -lam)
                        r2 = psm.tile([128, 1], f32, tag="r2", bufs=4)
                        nc.vector.reciprocal(out=r2[:], in_=s2[:])
                        cneg = psm.tile([128, 1], f32, tag="cneg", bufs=4)
                        nc.vector.scalar_tensor_tensor(
                            out=cneg[:], in0=s1[:], scalar=lamb[:, h:h + 1],
                            in1=r2[:], op0=MULT, op1=MULT)
                        # g0 = e1 + cneg*e2 ; g = relu(g0), accum D'
                        g0 = pse.tile([128, WMAX], bf, tag="g0", bufs=2)
                        g = pse.tile([128, WMAX], bf, tag="g", bufs=2)
                        dsum = psm.tile([128, 1], f32, tag="dsum", bufs=4)
                        nc.vector.scalar_tensor_tensor(
                            out=g0[:, 0:w], in0=e2[:, 0:w], scalar=cneg[:],
                            in1=e1[:, 0:w], op0=MULT, op1=ADD)
                        nc.vector.tensor_scalar(
                            out=g[:, 0:w], in0=g0[:, 0:w], scalar1=0.0,
                            scalar2=0.0, op0=MAX, op1=ADD, accum_out=dsum[:])
                        # recd = 1 / (D' + 1e-6 * s1); gn = g * recd
                        dtmp = psm.tile([128, 1], f32, tag="dtmp", bufs=4)
                        nc.vector.scalar_tensor_tensor(
                            out=dtmp[:], in0=s1[:], scalar=1e-6, in1=dsum[:],
                            op0=MULT, op1=ADD)
                        recd = psm.tile([128, 1], f32, tag="recd", bufs=4)
                        nc.vector.reciprocal(out=recd[:], in_=dtmp[:])
                        gn = pse.tile([128, WMAX], bf, tag="gn", bufs=2)
                        nc.vector.tensor_scalar(
                            out=gn[:, 0:w], in0=g[:, 0:w], scalar1=recd[:],
                            scalar2=0.0, op0=MULT, op1=ADD)

                        # transpose gn -> gT (PSUM) -> SBUF
                        ps_tr = psc.tile([128, kw, 128], bf, tag="trg", bufs=2)
                        for j in range(kw):
                            nc.tensor.transpose(ps_tr[:, j, :],
                                                gn[:, 128 * j:128 * (j + 1)], idb[:])
                        gts = pse.tile([128, 5, 128], bf, tag="gts", bufs=2)
                        nc.vector.tensor_copy(out=gts[:, 0:kw, :], in_=ps_tr[:])

                        # AV: out^T[hd, q] += v[k,:].T-contract over window
                        for j in range(kw):
                            nc.tensor.matmul(ps_av[h][:], vsm[:, kstart + j, :],
                                             gts[:, j, :],
                                             start=(j == 0), stop=(j == kw - 1))

                    nc.vector.tensor_copy(out=att[:, 0, qsl], in_=ps_av[0][:])
                    nc.vector.tensor_copy(out=att[:, 1, qsl], in_=ps_av[1][:])

            # ---- Phase 3: output projection (row-sharded Wo partial) ----
            with tc.tile_pool(name="po", bufs=1, space="PSUM") as po, \
                 tc.tile_pool(name="pos", bufs=1) as pos:
                for qi in range(NQT):
                    qsl = slice(qi * 128, (qi + 1) * 128)
                    for dch in range(4):
                        dsl = slice(dch * 512, (dch + 1) * 512)
                        ps_o = po.tile([128, 512], f32, tag="o", bufs=5)
                        nc.tensor.matmul(ps_o[:], att[:, 0, qsl], wo[:, 0, dsl],
                                         start=True, stop=False)
                        nc.tensor.matmul(ps_o[:], att[:, 1, qsl], wo[:, 1, dsl],
                                         start=False, stop=True)
                        so = pos.tile([128, 512], f16, tag="so", bufs=6)
                        nc.scalar.copy(out=so[:], in_=ps_o[:])
                        nc.sync.dma_start(out=out_d[qsl, dsl], in_=so[:])

    nc.compile()
    return nc


def get_program():
    if "nc" not in _CACHE:
        _CACHE["nc"] = _build_program()
    return _CACHE["nc"]


def _prep_inputs(x, Wq, Wk, Wv, Wo, lam):
    xt = np.ascontiguousarray(x.reshape(S, D).T.astype(BF)
                              .reshape(NKT, 128, S).transpose(1, 0, 2))
    in_maps = []
    for c in range(N_CORES):
        h0 = 2 * c
        kv = c // 2
        wq_c = np.ascontiguousarray(
            Wq[:, h0 * 128:(h0 + 2) * 128].astype(BF)
            .reshape(NKT, 128, 2, 128).transpose(1, 0, 2, 3))
        wk_c = np.ascontiguousarray(
            Wk[:, kv * 128:(kv + 1) * 128].astype(BF)
            .reshape(NKT, 128, 128).transpose(1, 0, 2))
        wv_c = np.ascontiguousarray(
            Wv[:, kv * 128:(kv + 1) * 128].astype(BF)
            .reshape(NKT, 128, 128).transpose(1, 0, 2))
        wo_c = np.ascontiguousarray(
            Wo[h0 * 128:(h0 + 2) * 128, :].astype(BF)
            .reshape(2, 128, D).transpose(1, 0, 2))
        lamn_c = np.array([[-float(lam[h0]), -float(lam[h0 + 1])]], dtype=np.float32)
        in_maps.append({"xt": xt, "wq": wq_c, "wk": wk_c, "wv": wv_c,
                        "wo": wo_c, "lamn": lamn_c})
    return in_maps


def kernel(x, Wq, Wk, Wv, Wo, lam):
    from concourse.bass_utils import run_bass_kernel_spmd

    nc = get_program()
    in_maps = _prep_inputs(np.asarray(x), np.asarray(Wq), np.asarray(Wk),
                           np.asarray(Wv), np.asarray(Wo), np.asarray(lam))
    res = run_bass_kernel_spmd(nc, in_maps, list(range(N_CORES)))
    out = np.zeros((S, D), dtype=np.float32)
    for c in range(N_CORES):
        out += res.results[c]["outp"].astype(np.float32)
    return out.reshape(1, S, D)



# revision 2
# speedup vs baseline: 1.2749x; 1.2749x over previous
"""Trainium2 Bass kernel for nn_ChimeraV2Block (dual-softmax differential
sliding-window attention block, B=1 S=2048 D=2048, 16 q-heads / 4 kv-heads,
head_dim 128, window 512).

Sharding: tensor-parallel over heads across 8 NeuronCores. Core c owns
q-heads {2c, 2c+1} and kv-head c//2 (GQA groups align with the split).
Wq/Wk/Wv column-sharded, Wo row-sharded; the 8 fp32 partial outputs are
summed on the host (the "all-reduce").

Design notes (v2): attention scores are computed TRANSPOSED (sT[k,q] =
K·Q^T per 128-tile of the sliding window) so the probability tensor is
k-major and the AV matmul needs no PE transposes. All PE work is genuine
matmuls (projections, scores, ones-vector row sums, AV, output proj),
emitted as one dense interleaved stream so the PE HAM clock stays at
2.4 GHz. Softmax row stats live as [1,128] free-dim vectors; per-q
scalars are broadcast across partitions on GpSimd. Edge masking is two
0/1 bf16 multiplies on the vector engine.
"""

import sys

if "/opt/trn_rl_repo" not in sys.path:
    sys.path.insert(0, "/opt/trn_rl_repo")

import numpy as np
import ml_dtypes

BF = ml_dtypes.bfloat16

S = 2048
D = 2048
H = 16
HK = 4
HD = 128
WIN = 512
THETA = 10000.0
N_CORES = 8
NQT = S // 128          # 16 q row-tiles
NKT = D // 128          # 16 contraction tiles for the projections
NEG = -1.0e30

_CACHE = {}


def _tables():
    """Shared RoPE tables [128, S] bf16 with head-dim-duplicated freqs
    (row p uses invf[p % 64]). sin table is sign-folded: the slice read
    for the lower-half output (rows 64:128) carries -sin so both output
    halves use a single ADD. Attention scale 1/8 is folded into Wq on
    the host."""
    invf = 1.0 / (THETA ** (np.arange(0, HD, 2, dtype=np.float64) / HD))  # [64]
    t = np.arange(S, dtype=np.float64)
    fr = np.outer(invf, t)  # [64, S]
    cosf = np.concatenate([np.cos(fr)] * 2, axis=0)
    sinf = np.concatenate([np.sin(fr), -np.sin(fr)], axis=0)
    return (np.ascontiguousarray(cosf, dtype=BF),
            np.ascontiguousarray(sinf, dtype=BF))


def _masks():
    """0/1 bf16 masks in the TRANSPOSED tile layout [k_loc, q_loc].
    m_low: window lower edge (j=0 tile, qi>=4): allowed iff k > q.
    m_diag: causal/diagonal tile (j=kw-1): allowed iff k <= q."""
    k = np.arange(128)[:, None]
    q = np.arange(128)[None, :]
    m_low = (k > q).astype(BF)
    m_diag = (k <= q).astype(BF)
    return np.ascontiguousarray(m_low), np.ascontiguousarray(m_diag)


def _build_program():
    import concourse.bacc as bacc
    import concourse.tile as tile
    from concourse import mybir

    bf = mybir.dt.bfloat16
    f32 = mybir.dt.float32
    f16 = mybir.dt.float16
    EXP = mybir.ActivationFunctionType.Exp
    MULT = mybir.AluOpType.mult
    ADD = mybir.AluOpType.add
    MAX = mybir.AluOpType.max

    nc = bacc.Bacc("TRN2", target_bir_lowering=False, debug=False,
                   num_devices=N_CORES)

    xt_d = nc.dram_tensor("xt", [128, NKT, S], bf, kind="ExternalInput")
    wq_d = nc.dram_tensor("wq", [128, NKT, 2, 128], bf, kind="ExternalInput")
    wk_d = nc.dram_tensor("wk", [128, NKT, 128], bf, kind="ExternalInput")
    wv_d = nc.dram_tensor("wv", [128, NKT, 128], bf, kind="ExternalInput")
    wo_d = nc.dram_tensor("wo", [128, 2, D], bf, kind="ExternalInput")
    lamn_d = nc.dram_tensor("lamn", [1, 2], f32, kind="ExternalInput")
    out_d = nc.dram_tensor("outp", [S, D], f16, kind="ExternalOutput")

    tabc_np, tabs_np = _tables()
    mlow_np, mdiag_np = _masks()
    tabc_d = nc.inline_tensor(tabc_np, "tab_c")
    tabs_d = nc.inline_tensor(tabs_np, "tab_s")
    mlow_d = nc.inline_tensor(mlow_np, "mask_low")
    mdiag_d = nc.inline_tensor(mdiag_np, "mask_diag")
    idb_d = nc.inline_tensor(np.eye(128, dtype=BF), "ident_bf")
    ones_d = nc.inline_tensor(np.ones((128, 1), dtype=BF), "ones_col")

    with tile.TileContext(nc) as tc:
        with tc.tile_pool(name="wp", bufs=1) as wp, \
             tc.tile_pool(name="sb", bufs=1) as sb, \
             tc.tile_pool(name="pbig", bufs=1, space="PSUM") as pbig, \
             tc.tile_pool(name="pst", bufs=1, space="PSUM") as pst, \
             tc.tile_pool(name="psm", bufs=1, space="PSUM") as psm, \
             tc.tile_pool(name="pav", bufs=1, space="PSUM") as pav:

            # ---- weights / tables / constants (DMA'd first) ----
            wq = wp.tile([128, NKT, 2, 128], bf)
            for i in range(4):
                nc.sync.dma_start(out=wq[:, 4 * i:4 * i + 4], in_=wq_d[:, 4 * i:4 * i + 4])
            wk = wp.tile([128, NKT, 128], bf)
            nc.sync.dma_start(out=wk[:], in_=wk_d[:])
            wv = wp.tile([128, NKT, 128], bf)
            nc.sync.dma_start(out=wv[:], in_=wv_d[:])
            tabc = wp.tile([128, S], bf)
            tabs = wp.tile([128, S], bf)
            nc.sync.dma_start(out=tabc[:], in_=tabc_d[:])
            nc.sync.dma_start(out=tabs[:], in_=tabs_d[:])
            mlow = wp.tile([128, 128], bf)
            nc.sync.dma_start(out=mlow[:], in_=mlow_d[:])
            mdiag = wp.tile([128, 128], bf)
            nc.sync.dma_start(out=mdiag[:], in_=mdiag_d[:])
            idb = wp.tile([128, 128], bf)
            nc.sync.dma_start(out=idb[:], in_=idb_d[:])
            ones1 = wp.tile([128, 1], bf)
            nc.sync.dma_start(out=ones1[:], in_=ones_d[:])
            lamn = wp.tile([1, 2], f32)
            nc.sync.dma_start(out=lamn[:], in_=lamn_d[:])
            wo = wp.tile([128, 2, D], bf)
            for i in range(4):
                nc.sync.dma_start(out=wo[:, :, 512 * i:512 * (i + 1)],
                                  in_=wo_d[:, :, 512 * i:512 * (i + 1)])
            xt = wp.tile([128, NKT, S], bf)
            for nch in range(4):
                sl = slice(nch * 512, (nch + 1) * 512)
                for kg in range(4):
                    nc.sync.dma_start(out=xt[:, 4 * kg:4 * kg + 4, sl],
                                      in_=xt_d[:, 4 * kg:4 * kg + 4, sl])

            # ---- persistent activations ----
            qt = wp.tile([128, 2, S], bf)       # RoPE'd q (wq pre-scaled), hd-major
            kt = wp.tile([128, S], bf)          # RoPE'd k, hd-major
            vsm = wp.tile([128, NQT, 128], bf)  # v, s-major [k_loc, hd]

            # ================= emission helpers =================

            def proj_chain(nch, t):
                """One projection accumulation chain (16 matmuls) for chunk
                nch, target t in {0:q0, 1:q1, 2:k, 3:v}, plus its RoPE or
                v-transpose epilogue."""
                sl = slice(nch * 512, (nch + 1) * 512)
                ps = pbig.tile([128, 512], f32, tag="big", bufs=2)
                if t == 0:
                    w_ap = wq[:, :, 0, :]
                elif t == 1:
                    w_ap = wq[:, :, 1, :]
                elif t == 2:
                    w_ap = wk
                else:
                    w_ap = wv
                for kti in range(NKT):
                    nc.tensor.matmul(ps[:], w_ap[:, kti, :], xt[:, kti, sl],
                                     start=(kti == 0), stop=(kti == NKT - 1))
                f = sb.tile([128, 512], bf, tag="ropef", bufs=3)
                nc.scalar.copy(out=f[:], in_=ps[:])
                if t < 3:
                    outt = qt[:, t, sl] if t < 2 else kt[:, sl]
                    m1 = sb.tile([128, 512], bf, tag="m1", bufs=2)
                    m2 = sb.tile([128, 512], bf, tag="m2", bufs=2)
                    nc.vector.tensor_mul(m1[:], f[:], tabc[:, sl])
                    nc.vector.tensor_mul(m2[0:64, :], f[64:128, :], tabs[64:128, sl])
                    nc.vector.tensor_mul(m2[64:128, :], f[0:64, :], tabs[0:64, sl])
                    nc.vector.tensor_add(outt, m1[:], m2[:])
                else:
                    ptv = pbig.tile([128, 4, 128], bf, tag="big", bufs=2)
                    for j in range(4):
                        nc.tensor.transpose(ptv[:, j, :],
                                            f[:, 128 * j:128 * (j + 1)], idb[:])
                    nc.vector.tensor_copy(out=vsm[:, 4 * nch:4 * (nch + 1), :],
                                          in_=ptv[:])

            # attention pipeline state carried between stages
            st = {}

            def kw_of(qi):
                return min(qi + 1, 5)

            def s1_emit(i):
                """Stage 1 for iteration i=(qi,h): transposed QK + exp + mask."""
                qi, h = i // 2, i % 2
                kw = kw_of(qi)
                jwA = min(kw, 3)
                g0 = max(0, qi - 4)
                qsl = slice(qi * 128, (qi + 1) * 128)
                e_all = sb.tile([128, 2, 5, 128], bf, tag="eall", bufs=3)
                for (joff, jw) in ((0, jwA), (jwA, kw - jwA)):
                    if jw == 0:
                        continue
                    sT = pst.tile([128, 2, 4, 128], f32, tag="sT", bufs=2)
                    for j in range(jw):
                        g = g0 + joff + j
                        ksl = slice(g * 128, (g + 1) * 128)
                        nc.tensor.matmul(sT[:, 0, j, :], kt[0:64, ksl],
                                         qt[0:64, h, qsl], start=True, stop=True)
                        nc.tensor.matmul(sT[:, 1, j, :], kt[64:128, ksl],
                                         qt[64:128, h, qsl], start=True, stop=True)
                    nc.scalar.activation(
                        out=e_all[:, :, joff:joff + jw, :],
                        in_=sT[:, :, 0:jw, :], func=EXP)
                # masks: j=0 lower-window edge (only when qi>=4), j=kw-1 diagonal
                if qi >= 4:
                    nc.vector.tensor_mul(
                        e_all[:, :, 0, :], e_all[:, :, 0, :],
                        mlow[:, None, :].broadcast_to([128, 2, 128]))
                nc.vector.tensor_mul(
                    e_all[:, :, kw - 1, :], e_all[:, :, kw - 1, :],
                    mdiag[:, None, :].broadcast_to([128, 2, 128]))
                st[("e", i)] = e_all

            def s2a_emit(i):
                """Stage 2a: ones-matmul sums of e1/e2 + cneg scalar chain."""
                qi, h = i // 2, i % 2
                kw = kw_of(qi)
                e_all = st.pop(("e", i))
                sm = psm.tile([1, 256], f32, tag="sm", bufs=1)
                for half in range(2):
                    for j in range(kw):
                        nc.tensor.matmul(sm[0:1, 128 * half:128 * (half + 1)],
                                         ones1[:], e_all[:, half, j, :],
                                         start=(j == 0), stop=(j == kw - 1))
                r2 = sb.tile([1, 128], f32, tag="r2", bufs=2)
                nc.vector.reciprocal_approx_fast(out=r2[:], in_=sm[0:1, 128:256])
                cneg = sb.tile([1, 128], f32, tag="cneg", bufs=2)
                nc.vector.scalar_tensor_tensor(
                    out=cneg[:], in0=sm[0:1, 0:128], scalar=lamn[0:1, h:h + 1],
                    in1=r2[:], op0=MULT, op1=MULT)
                cnegB = sb.tile([128, 128], f32, tag="cnegB", bufs=2)
                nc.gpsimd.partition_broadcast(cnegB[:], cneg[:])
                # gt = relu(e1 + cneg * e2)   (vector, k-major)
                gt = sb.tile([128, 5, 128], bf, tag="gt", bufs=2)
                nc.vector.tensor_mul(
                    gt[:, 0:kw, :], e_all[:, 1, 0:kw, :],
                    cnegB[:, None, :].broadcast_to([128, kw, 128]))
                nc.vector.tensor_add(gt[:, 0:kw, :], gt[:, 0:kw, :],
                                     e_all[:, 0, 0:kw, :])
                nc.vector.tensor_scalar(out=gt[:, 0:kw, :], in0=gt[:, 0:kw, :],
                                        scalar1=0.0, scalar2=0.0,
                                        op0=MAX, op1=ADD)
                st[("gt", i)] = gt

            def s2b_emit(i):
                """Stage 2b: dsum chain, AV chain, normalize into att."""
                qi, h = i // 2, i % 2
                kw = kw_of(qi)
                g0 = max(0, qi - 4)
                gt = st.pop(("gt", i))
                if h == 0:
                    av = pav.tile([128, 4, 128], f32, tag="av", bufs=1)
                    att = sb.tile([128, 2, 128], bf, tag="att", bufs=2)
                    st[("av", qi)] = av
                    st[("att", qi)] = att
                else:
                    av = st[("av", qi)]
                    att = st[("att", qi)]
                # dsum into av tile partition-0 row region [1,128] at slot 2+h
                nc.tensor.matmul(av[0:1, 2 + h, :], ones1[:], gt[:, 0, :],
                                 start=True, stop=(kw == 1))
                for j in range(1, kw):
                    nc.tensor.matmul(av[0:1, 2 + h, :], ones1[:], gt[:, j, :],
                                     start=False, stop=(j == kw - 1))
                for j in range(kw):
                    nc.tensor.matmul(av[:, h, :], vsm[:, g0 + j, :], gt[:, j, :],
                                     start=(j == 0), stop=(j == kw - 1))
                recd = sb.tile([1, 128], f32, tag="recd", bufs=2)
                nc.vector.reciprocal_approx_fast(out=recd[:], in_=av[0:1, 2 + h, :])
                recdB = sb.tile([128, 128], f32, tag="recdB", bufs=2)
                nc.gpsimd.partition_broadcast(recdB[:], recd[:])
                nc.vector.tensor_mul(att[:, h, :], av[:, h, :], recdB[:])

            def p3_emit(qi):
                """Output projection for one q-tile (row-sharded Wo partial)."""
                qsl = slice(qi * 128, (qi + 1) * 128)
                att = st.pop(("att", qi))
                st.pop(("av", qi))
                for dch in range(4):
                    dsl = slice(dch * 512, (dch + 1) * 512)
                    ps_o = pbig.tile([128, 512], f32, tag="big", bufs=2)
                    nc.tensor.matmul(ps_o[:], att[:, 0, :], wo[:, 0, dsl],
                                     start=True, stop=False)
                    nc.tensor.matmul(ps_o[:], att[:, 1, :], wo[:, 1, dsl],
                                     start=False, stop=True)
                    so = sb.tile([128, 512], f16, tag="so", bufs=6)
                    if dch % 2 == 0:
                        nc.scalar.copy(out=so[:], in_=ps_o[:])
                    else:
                        nc.vector.tensor_copy(out=so[:], in_=ps_o[:])
                    nc.sync.dma_start(out=out_d[qsl, dsl], in_=so[:])

            # ================= emission schedule =================
            for t in range(4):
                proj_chain(0, t)
            for t in range(4):
                proj_chain(1, t)
            proj_left = [(2, 0), (2, 1), (2, 2), (2, 3),
                         (3, 0), (3, 1), (3, 2), (3, 3)]
            NI = 32
            for i in range(NI + 2):
                if i < NI:
                    s1_emit(i)
                if 0 <= i - 1 < NI:
                    s2a_emit(i - 1)
                if 0 <= i - 2 < NI:
                    s2b_emit(i - 2)
                    if (i - 2) % 2 == 1:
                        p3_emit((i - 2) // 2)
                if proj_left and i % 2 == 1:
                    nch, t = proj_left.pop(0)
                    proj_chain(nch, t)

    nc.compile()
    return nc


def get_program():
    if "nc" not in _CACHE:
        _CACHE["nc"] = _build_program()
    return _CACHE["nc"]


def _prep_inputs(x, Wq, Wk, Wv, Wo, lam):
    xt = np.ascontiguousarray(x.reshape(S, D).T.astype(BF)
                              .reshape(NKT, 128, S).transpose(1, 0, 2))
    in_maps = []
    for c in range(N_CORES):
        h0 = 2 * c
        kv = c // 2
        # attention scale 1/sqrt(64) folded into Wq (exact pow2 in bf16)
        wq_c = np.ascontiguousarray(
            (Wq[:, h0 * 128:(h0 + 2) * 128] * 0.125).astype(BF)
            .reshape(NKT, 128, 2, 128).transpose(1, 0, 2, 3))
        wk_c = np.ascontiguousarray(
            Wk[:, kv * 128:(kv + 1) * 128].astype(BF)
            .reshape(NKT, 128, 128).transpose(1, 0, 2))
        wv_c = np.ascontiguousarray(
            Wv[:, kv * 128:(kv + 1) * 128].astype(BF)
            .reshape(NKT, 128, 128).transpose(1, 0, 2))
        wo_c = np.ascontiguousarray(
            Wo[h0 * 128:(h0 + 2) * 128, :].astype(BF)
            .reshape(2, 128, D).transpose(1, 0, 2))
        lamn_c = np.array([[-float(lam[h0]), -float(lam[h0 + 1])]], dtype=np.float32)
        in_maps.append({"xt": xt, "wq": wq_c, "wk": wk_c, "wv": wv_c,
                        "wo": wo_c, "lamn": lamn_c})
    return in_maps


def kernel(x, Wq, Wk, Wv, Wo, lam):
    from concourse.bass_utils import run_bass_kernel_spmd

    nc = get_program()
    in_maps = _prep_inputs(np.asarray(x), np.asarray(Wq), np.asarray(Wk),
                           np.asarray(Wv), np.asarray(Wo), np.asarray(lam))
    res = run_bass_kernel_spmd(nc, in_maps, list(range(N_CORES)))
    out = np.zeros((S, D), dtype=np.float32)
    for c in range(N_CORES):
        out += res.results[c]["outp"].astype(np.float32)
    return out.reshape(1, S, D)


# revision 7
# speedup vs baseline: 1.3604x; 1.0671x over previous
"""Trainium2 Bass kernel for nn_ChimeraV2Block (dual-softmax differential
sliding-window attention block, B=1 S=2048 D=2048, 16 q-heads / 4 kv-heads,
head_dim 128, window 512).

Sharding: tensor-parallel over heads across 8 NeuronCores. Core c owns
q-heads {2c, 2c+1} and kv-head c//2 (GQA groups align with the split).
Wq/Wk/Wv column-sharded, Wo row-sharded; the 8 fp32 partial outputs are
summed on the host (the "all-reduce").

Design notes (v2): attention scores are computed TRANSPOSED (sT[k,q] =
K·Q^T per 128-tile of the sliding window) so the probability tensor is
k-major and the AV matmul needs no PE transposes. All PE work is genuine
matmuls (projections, scores, ones-vector row sums, AV, output proj),
emitted as one dense interleaved stream so the PE HAM clock stays at
2.4 GHz. Softmax row stats live as [1,128] free-dim vectors; per-q
scalars are broadcast across partitions on GpSimd. Edge masking is two
0/1 bf16 multiplies on the vector engine.
"""

import sys

if "/opt/trn_rl_repo" not in sys.path:
    sys.path.insert(0, "/opt/trn_rl_repo")

import numpy as np
import ml_dtypes

BF = ml_dtypes.bfloat16

S = 2048
D = 2048
H = 16
HK = 4
HD = 128
WIN = 512
THETA = 10000.0
N_CORES = 8
NQT = S // 128          # 16 q row-tiles
NKT = D // 128          # 16 contraction tiles for the projections
NEG = -1.0e30

_CACHE = {}


def _tables():
    """Shared RoPE tables [128, S] bf16 with head-dim-duplicated freqs
    (row p uses invf[p % 64]). sin table is sign-folded: the slice read
    for the lower-half output (rows 64:128) carries -sin so both output
    halves use a single ADD. Attention scale 1/8 is folded into Wq on
    the host."""
    invf = 1.0 / (THETA ** (np.arange(0, HD, 2, dtype=np.float64) / HD))  # [64]
    t = np.arange(S, dtype=np.float64)
    fr = np.outer(invf, t)  # [64, S]
    cosf = np.concatenate([np.cos(fr)] * 2, axis=0)
    sinf = np.concatenate([np.sin(fr), -np.sin(fr)], axis=0)
    return (np.ascontiguousarray(cosf, dtype=BF),
            np.ascontiguousarray(sinf, dtype=BF))


def _masks():
    """0/1 bf16 masks in the TRANSPOSED tile layout [k_loc, q_loc].
    m_low: window lower edge (j=0 tile, qi>=4): allowed iff k > q.
    m_diag: causal/diagonal tile (j=kw-1): allowed iff k <= q."""
    k = np.arange(128)[:, None]
    q = np.arange(128)[None, :]
    m_low = (k > q).astype(BF)
    m_diag = (k <= q).astype(BF)
    return np.ascontiguousarray(m_low), np.ascontiguousarray(m_diag)


def _build_program():
    import concourse.bacc as bacc
    import concourse.tile as tile
    from concourse import mybir

    bf = mybir.dt.bfloat16
    f32 = mybir.dt.float32
    f16 = mybir.dt.float16
    EXP = mybir.ActivationFunctionType.Exp
    MULT = mybir.AluOpType.mult
    ADD = mybir.AluOpType.add
    MAX = mybir.AluOpType.max

    nc = bacc.Bacc("TRN2", target_bir_lowering=False, debug=False,
                   num_devices=N_CORES)

    xt_d = nc.dram_tensor("xt", [128, NKT, S], bf, kind="ExternalInput")
    wq_d = nc.dram_tensor("wq", [128, NKT, 2, 128], bf, kind="ExternalInput")
    wk_d = nc.dram_tensor("wk", [128, NKT, 128], bf, kind="ExternalInput")
    wv_d = nc.dram_tensor("wv", [128, NKT, 128], bf, kind="ExternalInput")
    wo_d = nc.dram_tensor("wo", [128, 2, D], bf, kind="ExternalInput")
    lamn_d = nc.dram_tensor("lamn", [1, 2], f32, kind="ExternalInput")
    out_d = nc.dram_tensor("outp", [S, D], f16, kind="ExternalOutput")

    tabc_np, tabs_np = _tables()
    mlow_np, mdiag_np = _masks()
    tabc_d = nc.inline_tensor(tabc_np, "tab_c")
    tabs_d = nc.inline_tensor(tabs_np, "tab_s")
    mlow_d = nc.inline_tensor(mlow_np, "mask_low")
    mdiag_d = nc.inline_tensor(mdiag_np, "mask_diag")
    idb_d = nc.inline_tensor(np.eye(128, dtype=BF), "ident_bf")
    ones_d = nc.inline_tensor(np.ones((128, 1), dtype=BF), "ones_col")
    onesr_d = nc.inline_tensor(np.ones((1, 128), dtype=BF), "ones_row")

    with tile.TileContext(nc) as tc:
        with tc.tile_pool(name="wp", bufs=1) as wp, \
             tc.tile_pool(name="sb", bufs=1) as sb, \
             tc.tile_pool(name="pbig", bufs=1, space="PSUM") as pbig, \
             tc.tile_pool(name="pst", bufs=1, space="PSUM") as pst, \
             tc.tile_pool(name="psm", bufs=1, space="PSUM") as psm, \
             tc.tile_pool(name="pav", bufs=1, space="PSUM") as pav:

            # ---- weights / tables / constants, ordered first-needed-first ----
            wq = wp.tile([128, NKT, 2, 128], bf)
            nc.sync.dma_start(out=wq[:], in_=wq_d[:])
            xt = wp.tile([128, NKT, S], bf)
            nc.sync.dma_start(out=xt[:, :, 0:512], in_=xt_d[:, :, 0:512])
            wk = wp.tile([128, NKT, 128], bf)
            nc.sync.dma_start(out=wk[:], in_=wk_d[:])
            wv = wp.tile([128, NKT, 128], bf)
            nc.sync.dma_start(out=wv[:], in_=wv_d[:])
            tabc = wp.tile([128, S], bf)
            tabs = wp.tile([128, S], bf)
            nc.sync.dma_start(out=tabc[:], in_=tabc_d[:])
            nc.sync.dma_start(out=tabs[:], in_=tabs_d[:])
            mlow = wp.tile([128, 128], bf)
            nc.sync.dma_start(out=mlow[:], in_=mlow_d[:])
            mdiag = wp.tile([128, 128], bf)
            nc.sync.dma_start(out=mdiag[:], in_=mdiag_d[:])
            idb = wp.tile([128, 128], bf)
            nc.sync.dma_start(out=idb[:], in_=idb_d[:])
            ones1 = wp.tile([128, 1], bf)
            nc.sync.dma_start(out=ones1[:], in_=ones_d[:])
            onesr = wp.tile([1, 128], bf)
            nc.sync.dma_start(out=onesr[:], in_=onesr_d[:])
            lamn = wp.tile([1, 2], f32)
            nc.sync.dma_start(out=lamn[:], in_=lamn_d[:])
            nc.sync.dma_start(out=xt[:, :, 512:1024], in_=xt_d[:, :, 512:1024])
            wo = wp.tile([128, 2, D], bf)
            nc.sync.dma_start(out=wo[:], in_=wo_d[:])
            nc.sync.dma_start(out=xt[:, :, 1024:1536], in_=xt_d[:, :, 1024:1536])
            nc.sync.dma_start(out=xt[:, :, 1536:2048], in_=xt_d[:, :, 1536:2048])

            # ---- persistent activations ----
            qt = wp.tile([128, 2, S], bf)       # RoPE'd q (wq pre-scaled), hd-major
            kt = wp.tile([128, S], bf)          # RoPE'd k, hd-major
            vsm = wp.tile([128, NQT, 128], bf)  # v, s-major [k_loc, hd]

            # ================= emission helpers =================

            def proj_chain(nch, t):
                """One projection accumulation chain (16 matmuls) for chunk
                nch, target t in {0:q0, 1:q1, 2:k, 3:v}, plus its RoPE or
                v-transpose epilogue."""
                sl = slice(nch * 512, (nch + 1) * 512)
                ps = pbig.tile([128, 512], f32, tag="big", bufs=2)
                if t == 0:
                    w_ap = wq[:, :, 0, :]
                elif t == 1:
                    w_ap = wq[:, :, 1, :]
                elif t == 2:
                    w_ap = wk
                else:
                    w_ap = wv
                for kti in range(NKT):
                    nc.tensor.matmul(ps[:], w_ap[:, kti, :], xt[:, kti, sl],
                                     start=(kti == 0), stop=(kti == NKT - 1))
                f = sb.tile([128, 512], bf, tag="ropef", bufs=3)
                nc.scalar.copy(out=f[:], in_=ps[:])
                if t < 3:
                    outt = qt[:, t, sl] if t < 2 else kt[:, sl]
                    m1 = sb.tile([128, 512], bf, tag="m1", bufs=2)
                    m2 = sb.tile([128, 512], bf, tag="m2", bufs=2)
                    nc.vector.tensor_mul(m1[:], f[:], tabc[:, sl])
                    nc.vector.tensor_mul(m2[0:64, :], f[64:128, :], tabs[64:128, sl])
                    nc.vector.tensor_mul(m2[64:128, :], f[0:64, :], tabs[0:64, sl])
                    nc.vector.tensor_add(outt, m1[:], m2[:])
                else:
                    ptv = pbig.tile([128, 4, 128], bf, tag="big", bufs=2)
                    for j in range(4):
                        nc.tensor.transpose(ptv[:, j, :],
                                            f[:, 128 * j:128 * (j + 1)], idb[:])
                    nc.vector.tensor_copy(out=vsm[:, 4 * nch:4 * (nch + 1), :],
                                          in_=ptv[:])

            # attention pipeline state carried between stages
            st = {}

            def kw_of(qi):
                return min(qi + 1, 5)

            def s1_emit(i):
                """Stage 1 for iteration i=(qi,h): transposed QK + exp + mask."""
                qi, h = i // 2, i % 2
                kw = kw_of(qi)
                jwA = min(kw, 3)
                g0 = max(0, qi - 4)
                qsl = slice(qi * 128, (qi + 1) * 128)
                e_all = sb.tile([128, 2, 5, 128], bf, tag="eall", bufs=3)
                for (joff, jw) in ((0, jwA), (jwA, kw - jwA)):
                    if jw == 0:
                        continue
                    sT = pst.tile([128, 2, 4, 128], f32, tag="sT", bufs=2)
                    for j in range(jw):
                        g = g0 + joff + j
                        ksl = slice(g * 128, (g + 1) * 128)
                        nc.tensor.matmul(sT[:, 0, j, :], kt[0:64, ksl],
                                         qt[0:64, h, qsl], start=True, stop=True)
                        nc.tensor.matmul(sT[:, 1, j, :], kt[64:128, ksl],
                                         qt[64:128, h, qsl], start=True, stop=True)
                    nc.scalar.activation(
                        out=e_all[:, :, joff:joff + jw, :],
                        in_=sT[:, :, 0:jw, :], func=EXP)
                # masks: j=0 lower-window edge (only when qi>=4), j=kw-1 diagonal
                if qi >= 4:
                    nc.vector.tensor_mul(
                        e_all[:, :, 0, :], e_all[:, :, 0, :],
                        mlow[:, None, :].broadcast_to([128, 2, 128]))
                nc.vector.tensor_mul(
                    e_all[:, :, kw - 1, :], e_all[:, :, kw - 1, :],
                    mdiag[:, None, :].broadcast_to([128, 2, 128]))
                st[("e", i)] = e_all

            def s2a_emit(i):
                """Stage 2a: ones-matmul sums of e1/e2, cneg scalar chain
                (with the per-partition broadcast done by a K=1 matmul), and
                the gt = relu(e1 + cneg*e2) combine."""
                qi, h = i // 2, i % 2
                kw = kw_of(qi)
                e_all = st.pop(("e", i))
                # sm2 bank layout: [0:1, 0:256] = s1|s2 sums,
                #                  [:, 256:384] = cnegB broadcast
                sm = psm.tile([128, 384], f32, tag="sm", bufs=1)
                for j in range(kw):
                    nc.tensor.matmul(sm[0:1, 0:256],
                                     ones1[:], e_all[:, :, j, :],
                                     start=(j == 0), stop=(j == kw - 1))
                r2 = sb.tile([1, 128], f32, tag="r2", bufs=3)
                nc.vector.reciprocal_approx_fast(out=r2[:], in_=sm[0:1, 128:256])
                cneg = sb.tile([1, 128], bf, tag="cneg", bufs=3)
                nc.vector.scalar_tensor_tensor(
                    out=cneg[:], in0=sm[0:1, 0:128], scalar=lamn[0:1, h:h + 1],
                    in1=r2[:], op0=MULT, op1=MULT)
                nc.tensor.matmul(sm[:, 256:384], onesr[:], cneg[:],
                                 start=True, stop=True)
                # gt = relu(e1 + cneg * e2)   (vector, k-major)
                gt = sb.tile([128, 5, 128], bf, tag="gt", bufs=3)
                nc.vector.tensor_mul(
                    gt[:, 0:kw, :], e_all[:, 1, 0:kw, :],
                    sm[:, None, 256:384].broadcast_to([128, kw, 128]))
                nc.vector.tensor_add(gt[:, 0:kw, :], gt[:, 0:kw, :],
                                     e_all[:, 0, 0:kw, :])
                nc.vector.tensor_scalar(out=gt[:, 0:kw, :], in0=gt[:, 0:kw, :],
                                        scalar1=0.0, scalar2=0.0,
                                        op0=MAX, op1=ADD)
                st[("gt", i)] = gt

            def s2b_emit(i):
                """Stage 2b: dsum chain, AV chain, normalize into att."""
                qi, h = i // 2, i % 2
                kw = kw_of(qi)
                g0 = max(0, qi - 4)
                gt = st.pop(("gt", i))
                if h == 0:
                    av = pav.tile([128, 4, 128], f32, tag="av", bufs=1)
                    att = sb.tile([128, 2, 128], bf, tag="att", bufs=2)
                    st[("av", qi)] = av
                    st[("att", qi)] = att
                else:
                    av = st[("av", qi)]
                    att = st[("att", qi)]
                # dsum into av tile partition-0 row region [1,128] at slot 2+h
                nc.tensor.matmul(av[0:1, 2 + h, :], ones1[:], gt[:, 0, :],
                                 start=True, stop=(kw == 1))
                for j in range(1, kw):
                    nc.tensor.matmul(av[0:1, 2 + h, :], ones1[:], gt[:, j, :],
                                     start=False, stop=(j == kw - 1))
                for j in range(kw):
                    nc.tensor.matmul(av[:, h, :], vsm[:, g0 + j, :], gt[:, j, :],
                                     start=(j == 0), stop=(j == kw - 1))
                recd = sb.tile([1, 128], f32, tag="recd", bufs=2)
                nc.vector.reciprocal_approx_fast(out=recd[:], in_=av[0:1, 2 + h, :])
                recdB = sb.tile([128, 128], f32, tag="recdB", bufs=2)
                nc.gpsimd.partition_broadcast(recdB[:], recd[:])
                nc.vector.tensor_mul(att[:, h, :], av[:, h, :], recdB[:])

            def p3_emit(qi):
                """Output projection for one q-tile (row-sharded Wo partial)."""
                qsl = slice(qi * 128, (qi + 1) * 128)
                att = st.pop(("att", qi))
                st.pop(("av", qi))
                for dch in range(4):
                    dsl = slice(dch * 512, (dch + 1) * 512)
                    ps_o = pbig.tile([128, 512], f32, tag="big", bufs=2)
                    nc.tensor.matmul(ps_o[:], att[:, 0, :], wo[:, 0, dsl],
                                     start=True, stop=False)
                    nc.tensor.matmul(ps_o[:], att[:, 1, :], wo[:, 1, dsl],
                                     start=False, stop=True)
                    so = sb.tile([128, 512], f16, tag="so", bufs=6)
                    nc.any.tensor_copy(out=so[:], in_=ps_o[:])
                    nc.sync.dma_start(out=out_d[qsl, dsl], in_=so[:])

            # ================= emission schedule =================
            for t in range(4):
                proj_chain(0, t)
            for t in range(4):
                proj_chain(1, t)
            proj_left = [(2, 0), (2, 1), (2, 2), (2, 3),
                         (3, 0), (3, 1), (3, 2), (3, 3)]
            NI = 32
            for i in range(NI + 3):
                if i < NI:
                    s1_emit(i)
                if 0 <= i - 1 < NI:
                    s2a_emit(i - 1)
                if 0 <= i - 3 < NI:
                    s2b_emit(i - 3)
                    if (i - 3) % 2 == 1:
                        p3_emit((i - 3) // 2)
                if proj_left and i % 2 == 1:
                    nch, t = proj_left.pop(0)
                    proj_chain(nch, t)

    nc.compile()
    return nc


def get_program():
    if "nc" not in _CACHE:
        _CACHE["nc"] = _build_program()
    return _CACHE["nc"]


def _prep_inputs(x, Wq, Wk, Wv, Wo, lam):
    xt = np.ascontiguousarray(x.reshape(S, D).T.astype(BF)
                              .reshape(NKT, 128, S).transpose(1, 0, 2))
    in_maps = []
    for c in range(N_CORES):
        h0 = 2 * c
        kv = c // 2
        # attention scale 1/sqrt(64) folded into Wq (exact pow2 in bf16)
        wq_c = np.ascontiguousarray(
            (Wq[:, h0 * 128:(h0 + 2) * 128] * 0.125).astype(BF)
            .reshape(NKT, 128, 2, 128).transpose(1, 0, 2, 3))
        wk_c = np.ascontiguousarray(
            Wk[:, kv * 128:(kv + 1) * 128].astype(BF)
            .reshape(NKT, 128, 128).transpose(1, 0, 2))
        wv_c = np.ascontiguousarray(
            Wv[:, kv * 128:(kv + 1) * 128].astype(BF)
            .reshape(NKT, 128, 128).transpose(1, 0, 2))
        wo_c = np.ascontiguousarray(
            Wo[h0 * 128:(h0 + 2) * 128, :].astype(BF)
            .reshape(2, 128, D).transpose(1, 0, 2))
        lamn_c = np.array([[-float(lam[h0]), -float(lam[h0 + 1])]], dtype=np.float32)
        in_maps.append({"xt": xt, "wq": wq_c, "wk": wk_c, "wv": wv_c,
                        "wo": wo_c, "lamn": lamn_c})
    return in_maps


def kernel(x, Wq, Wk, Wv, Wo, lam):
    from concourse.bass_utils import run_bass_kernel_spmd

    nc = get_program()
    in_maps = _prep_inputs(np.asarray(x), np.asarray(Wq), np.asarray(Wk),
                           np.asarray(Wv), np.asarray(Wo), np.asarray(lam))
    res = run_bass_kernel_spmd(nc, in_maps, list(range(N_CORES)))
    out = np.zeros((S, D), dtype=np.float32)
    for c in range(N_CORES):
        out += res.results[c]["outp"].astype(np.float32)
    return out.reshape(1, S, D)


# revision 12
# speedup vs baseline: 1.3764x; 1.0117x over previous
"""Trainium2 Bass kernel for nn_ChimeraV2Block (dual-softmax differential
sliding-window attention block, B=1 S=2048 D=2048, 16 q-heads / 4 kv-heads,
head_dim 128, window 512).

Sharding: tensor-parallel over heads across 8 NeuronCores. Core c owns
q-heads {2c, 2c+1} and kv-head c//2 (GQA groups align with the split).
Wq/Wk/Wv column-sharded, Wo row-sharded; the 8 fp32 partial outputs are
summed on the host (the "all-reduce").

Design notes (v2): attention scores are computed TRANSPOSED (sT[k,q] =
K·Q^T per 128-tile of the sliding window) so the probability tensor is
k-major and the AV matmul needs no PE transposes. All PE work is genuine
matmuls (projections, scores, ones-vector row sums, AV, output proj),
emitted as one dense interleaved stream so the PE HAM clock stays at
2.4 GHz. Softmax row stats live as [1,128] free-dim vectors; per-q
scalars are broadcast across partitions on GpSimd. Edge masking is two
0/1 bf16 multiplies on the vector engine.
"""

import sys

if "/opt/trn_rl_repo" not in sys.path:
    sys.path.insert(0, "/opt/trn_rl_repo")

import numpy as np
import ml_dtypes

BF = ml_dtypes.bfloat16

S = 2048
D = 2048
H = 16
HK = 4
HD = 128
WIN = 512
THETA = 10000.0
N_CORES = 8
NQT = S // 128          # 16 q row-tiles
NKT = D // 128          # 16 contraction tiles for the projections
NEG = -1.0e30

_CACHE = {}


def _tables():
    """Shared RoPE tables [128, S] bf16 with head-dim-duplicated freqs
    (row p uses invf[p % 64]). sin table is sign-folded: the slice read
    for the lower-half output (rows 64:128) carries -sin so both output
    halves use a single ADD. Attention scale 1/8 is folded into Wq on
    the host."""
    invf = 1.0 / (THETA ** (np.arange(0, HD, 2, dtype=np.float64) / HD))  # [64]
    t = np.arange(S, dtype=np.float64)
    fr = np.outer(invf, t)  # [64, S]
    cosf = np.concatenate([np.cos(fr)] * 2, axis=0)
    sinf = np.concatenate([np.sin(fr), -np.sin(fr)], axis=0)
    return (np.ascontiguousarray(cosf, dtype=BF),
            np.ascontiguousarray(sinf, dtype=BF))


def _masks():
    """0/1 bf16 masks in the TRANSPOSED tile layout [k_loc, q_loc].
    m_low: window lower edge (j=0 tile, qi>=4): allowed iff k > q.
    m_diag: causal/diagonal tile (j=kw-1): allowed iff k <= q."""
    k = np.arange(128)[:, None]
    q = np.arange(128)[None, :]
    m_low = (k > q).astype(BF)
    m_diag = (k <= q).astype(BF)
    return np.ascontiguousarray(m_low), np.ascontiguousarray(m_diag)


def _build_program():
    import concourse.bacc as bacc
    import concourse.tile as tile
    from concourse import mybir

    bf = mybir.dt.bfloat16
    f32 = mybir.dt.float32
    f16 = mybir.dt.float16
    EXP = mybir.ActivationFunctionType.Exp
    MULT = mybir.AluOpType.mult
    ADD = mybir.AluOpType.add
    MAX = mybir.AluOpType.max

    nc = bacc.Bacc("TRN2", target_bir_lowering=False, debug=False,
                   num_devices=N_CORES)

    xt_d = nc.dram_tensor("xt", [128, NKT, S], bf, kind="ExternalInput")
    wq_d = nc.dram_tensor("wq", [128, NKT, 2, 128], bf, kind="ExternalInput")
    wk_d = nc.dram_tensor("wk", [128, NKT, 128], bf, kind="ExternalInput")
    wv_d = nc.dram_tensor("wv", [128, NKT, 128], bf, kind="ExternalInput")
    wo_d = nc.dram_tensor("wo", [128, 2, D], bf, kind="ExternalInput")
    lamn_d = nc.dram_tensor("lamn", [1, 2], f32, kind="ExternalInput")
    out_d = nc.dram_tensor("outp", [S, D], f16, kind="ExternalOutput")

    tabc_np, tabs_np = _tables()
    mlow_np, mdiag_np = _masks()
    tabc_d = nc.inline_tensor(tabc_np, "tab_c")
    tabs_d = nc.inline_tensor(tabs_np, "tab_s")
    mlow_d = nc.inline_tensor(mlow_np, "mask_low")
    mdiag_d = nc.inline_tensor(mdiag_np, "mask_diag")
    idb_d = nc.inline_tensor(np.eye(128, dtype=BF), "ident_bf")
    ones_d = nc.inline_tensor(np.ones((128, 1), dtype=BF), "ones_col")
    onesr_d = nc.inline_tensor(np.ones((1, 128), dtype=BF), "ones_row")

    with tile.TileContext(nc) as tc:
        with tc.tile_pool(name="wp", bufs=1) as wp, \
             tc.tile_pool(name="sb", bufs=1) as sb, \
             tc.tile_pool(name="pbig", bufs=1, space="PSUM") as pbig, \
             tc.tile_pool(name="pst", bufs=1, space="PSUM") as pst, \
             tc.tile_pool(name="psm", bufs=1, space="PSUM") as psm, \
             tc.tile_pool(name="pav", bufs=1, space="PSUM") as pav:

            # ---- weights / tables / constants, ordered first-needed-first ----
            wq = wp.tile([128, NKT, 2, 128], bf)
            nc.sync.dma_start(out=wq[:, 0:8], in_=wq_d[:, 0:8])
            xt = wp.tile([128, NKT, S], bf)
            nc.sync.dma_start(out=xt[:, 0:4, 0:512], in_=xt_d[:, 0:4, 0:512])
            nc.sync.dma_start(out=wq[:, 8:16], in_=wq_d[:, 8:16])
            nc.sync.dma_start(out=xt[:, 4:8, 0:512], in_=xt_d[:, 4:8, 0:512])
            nc.sync.dma_start(out=xt[:, 8:16, 0:512], in_=xt_d[:, 8:16, 0:512])
            wk = wp.tile([128, NKT, 128], bf)
            nc.sync.dma_start(out=wk[:], in_=wk_d[:])
            wv = wp.tile([128, NKT, 128], bf)
            nc.sync.dma_start(out=wv[:], in_=wv_d[:])
            tabc = wp.tile([128, S], bf)
            tabs = wp.tile([128, S], bf)
            nc.sync.dma_start(out=tabc[:], in_=tabc_d[:])
            nc.sync.dma_start(out=tabs[:], in_=tabs_d[:])
            mlow = wp.tile([128, 128], bf)
            nc.sync.dma_start(out=mlow[:], in_=mlow_d[:])
            mdiag = wp.tile([128, 128], bf)
            nc.sync.dma_start(out=mdiag[:], in_=mdiag_d[:])
            idb = wp.tile([128, 128], bf)
            nc.sync.dma_start(out=idb[:], in_=idb_d[:])
            ones1 = wp.tile([128, 1], bf)
            nc.sync.dma_start(out=ones1[:], in_=ones_d[:])
            onesr = wp.tile([1, 128], bf)
            nc.sync.dma_start(out=onesr[:], in_=onesr_d[:])
            lamn = wp.tile([1, 2], f32)
            nc.sync.dma_start(out=lamn[:], in_=lamn_d[:])
            nc.sync.dma_start(out=xt[:, :, 512:1024], in_=xt_d[:, :, 512:1024])
            wo = wp.tile([128, 2, D], bf)
            nc.sync.dma_start(out=wo[:], in_=wo_d[:])
            nc.sync.dma_start(out=xt[:, :, 1024:1536], in_=xt_d[:, :, 1024:1536])
            nc.sync.dma_start(out=xt[:, :, 1536:2048], in_=xt_d[:, :, 1536:2048])

            # ---- persistent activations ----
            qt = wp.tile([128, 2, S], bf)       # RoPE'd q (wq pre-scaled), hd-major
            kt = wp.tile([128, S], bf)          # RoPE'd k, hd-major
            vsm = wp.tile([128, NQT, 128], bf)  # v, s-major [k_loc, hd]

            # ================= emission helpers =================

            def proj_chain(nch, t):
                """One projection accumulation chain (16 matmuls) for chunk
                nch, target t in {0:q0, 1:q1, 2:k, 3:v}, plus its RoPE or
                v-transpose epilogue."""
                sl = slice(nch * 512, (nch + 1) * 512)
                ps = pbig.tile([128, 512], f32, tag="big", bufs=2)
                if t == 0:
                    w_ap = wq[:, :, 0, :]
                elif t == 1:
                    w_ap = wq[:, :, 1, :]
                elif t == 2:
                    w_ap = wk
                else:
                    w_ap = wv
                for kti in range(NKT):
                    nc.tensor.matmul(ps[:], w_ap[:, kti, :], xt[:, kti, sl],
                                     start=(kti == 0), stop=(kti == NKT - 1))
                f = sb.tile([128, 512], bf, tag="ropef", bufs=3)
                nc.scalar.copy(out=f[:], in_=ps[:])
                if t < 3:
                    outt = qt[:, t, sl] if t < 2 else kt[:, sl]
                    m1 = sb.tile([128, 512], bf, tag="m1", bufs=2)
                    m2 = sb.tile([128, 512], bf, tag="m2", bufs=2)
                    nc.vector.tensor_mul(m1[:], f[:], tabc[:, sl])
                    nc.vector.tensor_mul(m2[0:64, :], f[64:128, :], tabs[64:128, sl])
                    nc.vector.tensor_mul(m2[64:128, :], f[0:64, :], tabs[0:64, sl])
                    nc.vector.tensor_add(outt, m1[:], m2[:])
                else:
                    ptv = pbig.tile([128, 4, 128], bf, tag="big", bufs=2)
                    for j in range(4):
                        nc.tensor.transpose(ptv[:, j, :],
                                            f[:, 128 * j:128 * (j + 1)], idb[:])
                    nc.vector.tensor_copy(out=vsm[:, 4 * nch:4 * (nch + 1), :],
                                          in_=ptv[:])

            # attention pipeline state carried between stages
            st = {}

            def kw_of(qi):
                return min(qi + 1, 5)

            def s1_emit(i):
                """Stage 1 for iteration i=(qi,h): transposed QK + exp + mask."""
                qi, h = i // 2, i % 2
                kw = kw_of(qi)
                jwA = min(kw, 3)
                g0 = max(0, qi - 4)
                qsl = slice(qi * 128, (qi + 1) * 128)
                # j-major layout so the per-j sum matmuls get a contiguous
                # [128, 256] moving operand
                e_all = sb.tile([128, 5, 2, 128], bf, tag="eall", bufs=3)
                for (joff, jw) in ((0, jwA), (jwA, kw - jwA)):
                    if jw == 0:
                        continue
                    sT = pst.tile([128, 2, 4, 128], f32, tag="sT", bufs=2)
                    for j in range(jw):
                        g = g0 + joff + j
                        ksl = slice(g * 128, (g + 1) * 128)
                        nc.tensor.matmul(sT[:, 0, j, :], kt[0:64, ksl],
                                         qt[0:64, h, qsl], start=True, stop=True)
                        nc.tensor.matmul(sT[:, 1, j, :], kt[64:128, ksl],
                                         qt[64:128, h, qsl], start=True, stop=True)
                    nc.scalar.activation(
                        out=e_all[:, joff:joff + jw, :, :].transpose([0, 2, 1, 3]),
                        in_=sT[:, :, 0:jw, :], func=EXP)
                # masks: j=0 lower-window edge (only when qi>=4), j=kw-1 diagonal
                if qi >= 4:
                    nc.vector.tensor_mul(
                        e_all[:, 0, :, :], e_all[:, 0, :, :],
                        mlow[:, None, :].broadcast_to([128, 2, 128]))
                nc.vector.tensor_mul(
                    e_all[:, kw - 1, :, :], e_all[:, kw - 1, :, :],
                    mdiag[:, None, :].broadcast_to([128, 2, 128]))
                st[("e", i)] = e_all

            def s2a_emit(i):
                """Stage 2a: ones-matmul sums of e1/e2, cneg scalar chain
                (with the per-partition broadcast done by a K=1 matmul), and
                the gt = relu(e1 + cneg*e2) combine."""
                qi, h = i // 2, i % 2
                kw = kw_of(qi)
                e_all = st.pop(("e", i))
                # sm2 bank layout: [0:1, 0:256] = s1|s2 sums,
                #                  [:, 256:384] = cnegB broadcast
                sm = psm.tile([128, 384], f32, tag="sm", bufs=1)
                for j in range(kw):
                    nc.tensor.matmul(sm[0:1, 0:256],
                                     ones1[:], e_all[:, j, :, :],
                                     start=(j == 0), stop=(j == kw - 1))
                r2 = sb.tile([1, 128], f32, tag="r2", bufs=3)
                nc.vector.reciprocal_approx_fast(out=r2[:], in_=sm[0:1, 128:256])
                cneg = sb.tile([1, 128], bf, tag="cneg", bufs=3)
                nc.vector.scalar_tensor_tensor(
                    out=cneg[:], in0=sm[0:1, 0:128], scalar=lamn[0:1, h:h + 1],
                    in1=r2[:], op0=MULT, op1=MULT)
                nc.tensor.matmul(sm[:, 256:384], onesr[:], cneg[:],
                                 start=True, stop=True)
                # gt = relu(e1 + cneg * e2)   (vector, k-major)
                gt = sb.tile([128, 5, 128], bf, tag="gt", bufs=3)
                nc.vector.tensor_mul(
                    gt[:, 0:kw, :], e_all[:, 0:kw, 1, :],
                    sm[:, None, 256:384].broadcast_to([128, kw, 128]))
                nc.vector.tensor_add(gt[:, 0:kw, :], gt[:, 0:kw, :],
                                     e_all[:, 0:kw, 0, :])
                nc.vector.tensor_scalar(out=gt[:, 0:kw, :], in0=gt[:, 0:kw, :],
                                        scalar1=0.0, scalar2=0.0,
                                        op0=MAX, op1=ADD)
                st[("gt", i)] = gt

            def s2b_emit(i):
                """Stage 2b: dsum chain, AV chain, normalize into att."""
                qi, h = i // 2, i % 2
                kw = kw_of(qi)
                g0 = max(0, qi - 4)
                gt = st.pop(("gt", i))
                if h == 0:
                    av = pav.tile([128, 4, 128], f32, tag="av", bufs=1)
                    att = sb.tile([128, 2, 128], bf, tag="att", bufs=2)
                    st[("av", qi)] = av
                    st[("att", qi)] = att
                else:
                    av = st[("av", qi)]
                    att = st[("att", qi)]
                # dsum into av tile partition-0 row region [1,128] at slot 2+h
                nc.tensor.matmul(av[0:1, 2 + h, :], ones1[:], gt[:, 0, :],
                                 start=True, stop=(kw == 1))
                for j in range(1, kw):
                    nc.tensor.matmul(av[0:1, 2 + h, :], ones1[:], gt[:, j, :],
                                     start=False, stop=(j == kw - 1))
                for j in range(kw):
                    nc.tensor.matmul(av[:, h, :], vsm[:, g0 + j, :], gt[:, j, :],
                                     start=(j == 0), stop=(j == kw - 1))
                recd = sb.tile([1, 128], f32, tag="recd", bufs=2)
                nc.vector.reciprocal_approx_fast(out=recd[:], in_=av[0:1, 2 + h, :])
                recdB = sb.tile([128, 128], f32, tag="recdB", bufs=2)
                nc.gpsimd.partition_broadcast(recdB[:], recd[:])
                nc.vector.tensor_mul(att[:, h, :], av[:, h, :], recdB[:])

            def p3_emit(qi):
                """Output projection for one q-tile (row-sharded Wo partial)."""
                qsl = slice(qi * 128, (qi + 1) * 128)
                att = st.pop(("att", qi))
                st.pop(("av", qi))
                for dch in range(4):
                    dsl = slice(dch * 512, (dch + 1) * 512)
                    ps_o = pbig.tile([128, 512], f32, tag="big", bufs=2)
                    nc.tensor.matmul(ps_o[:], att[:, 0, :], wo[:, 0, dsl],
                                     start=True, stop=False)
                    nc.tensor.matmul(ps_o[:], att[:, 1, :], wo[:, 1, dsl],
                                     start=False, stop=True)
                    so = sb.tile([128, 512], f16, tag="so", bufs=6)
                    nc.any.tensor_copy(out=so[:], in_=ps_o[:])
                    nc.sync.dma_start(out=out_d[qsl, dsl], in_=so[:])

            # ================= emission schedule =================
            for t in range(4):
                proj_chain(0, t)
            for t in range(4):
                proj_chain(1, t)
            proj_left = [(2, 0), (2, 1), (2, 2), (2, 3),
                         (3, 0), (3, 1), (3, 2), (3, 3)]
            NI = 32
            for i in range(NI + 3):
                if i < NI:
                    s1_emit(i)
                if 0 <= i - 1 < NI:
                    s2a_emit(i - 1)
                if 0 <= i - 3 < NI:
                    s2b_emit(i - 3)
                    if (i - 3) % 2 == 1:
                        p3_emit((i - 3) // 2)
                if proj_left and i % 2 == 1:
                    nch, t = proj_left.pop(0)
                    proj_chain(nch, t)

    nc.compile()
    return nc


def get_program():
    if "nc" not in _CACHE:
        _CACHE["nc"] = _build_program()
    return _CACHE["nc"]


def _prep_inputs(x, Wq, Wk, Wv, Wo, lam):
    xt = np.ascontiguousarray(x.reshape(S, D).T.astype(BF)
                              .reshape(NKT, 128, S).transpose(1, 0, 2))
    in_maps = []
    for c in range(N_CORES):
        h0 = 2 * c
        kv = c // 2
        # attention scale 1/sqrt(64) folded into Wq (exact pow2 in bf16)
        wq_c = np.ascontiguousarray(
            (Wq[:, h0 * 128:(h0 + 2) * 128] * 0.125).astype(BF)
            .reshape(NKT, 128, 2, 128).transpose(1, 0, 2, 3))
        wk_c = np.ascontiguousarray(
            Wk[:, kv * 128:(kv + 1) * 128].astype(BF)
            .reshape(NKT, 128, 128).transpose(1, 0, 2))
        wv_c = np.ascontiguousarray(
            Wv[:, kv * 128:(kv + 1) * 128].astype(BF)
            .reshape(NKT, 128, 128).transpose(1, 0, 2))
        wo_c = np.ascontiguousarray(
            Wo[h0 * 128:(h0 + 2) * 128, :].astype(BF)
            .reshape(2, 128, D).transpose(1, 0, 2))
        lamn_c = np.array([[-float(lam[h0]), -float(lam[h0 + 1])]], dtype=np.float32)
        in_maps.append({"xt": xt, "wq": wq_c, "wk": wk_c, "wv": wv_c,
                        "wo": wo_c, "lamn": lamn_c})
    return in_maps


def kernel(x, Wq, Wk, Wv, Wo, lam):
    from concourse.bass_utils import run_bass_kernel_spmd

    nc = get_program()
    in_maps = _prep_inputs(np.asarray(x), np.asarray(Wq), np.asarray(Wk),
                           np.asarray(Wv), np.asarray(Wo), np.asarray(lam))
    res = run_bass_kernel_spmd(nc, in_maps, list(range(N_CORES)))
    out = np.zeros((S, D), dtype=np.float32)
    for c in range(N_CORES):
        out += res.results[c]["outp"].astype(np.float32)
    return out.reshape(1, S, D)


# revision 18
# speedup vs baseline: 1.3899x; 1.0098x over previous
"""Trainium2 Bass kernel for nn_ChimeraV2Block (dual-softmax differential
sliding-window attention block, B=1 S=2048 D=2048, 16 q-heads / 4 kv-heads,
head_dim 128, window 512).

Sharding: tensor-parallel over heads across 8 NeuronCores. Core c owns
q-heads {2c, 2c+1} and kv-head c//2 (GQA groups align with the split).
Wq/Wk/Wv column-sharded, Wo row-sharded; the 8 fp32 partial outputs are
summed on the host (the "all-reduce").

Design notes (v2): attention scores are computed TRANSPOSED (sT[k,q] =
K·Q^T per 128-tile of the sliding window) so the probability tensor is
k-major and the AV matmul needs no PE transposes. All PE work is genuine
matmuls (projections, scores, ones-vector row sums, AV, output proj),
emitted as one dense interleaved stream so the PE HAM clock stays at
2.4 GHz. Softmax row stats live as [1,128] free-dim vectors; per-q
scalars are broadcast across partitions on GpSimd. Edge masking is two
0/1 bf16 multiplies on the vector engine.
"""

import sys

if "/opt/trn_rl_repo" not in sys.path:
    sys.path.insert(0, "/opt/trn_rl_repo")

import numpy as np
import ml_dtypes

BF = ml_dtypes.bfloat16

S = 2048
D = 2048
H = 16
HK = 4
HD = 128
WIN = 512
THETA = 10000.0
N_CORES = 8
NQT = S // 128          # 16 q row-tiles
NKT = D // 128          # 16 contraction tiles for the projections
NEG = -1.0e30

_CACHE = {}


def _tables():
    """Shared RoPE tables [128, S] bf16 with head-dim-duplicated freqs
    (row p uses invf[p % 64]). sin table is sign-folded: the slice read
    for the lower-half output (rows 64:128) carries -sin so both output
    halves use a single ADD. Attention scale 1/8 is folded into Wq on
    the host."""
    invf = 1.0 / (THETA ** (np.arange(0, HD, 2, dtype=np.float64) / HD))  # [64]
    t = np.arange(S, dtype=np.float64)
    fr = np.outer(invf, t)  # [64, S]
    cosf = np.concatenate([np.cos(fr)] * 2, axis=0)
    sinf = np.concatenate([np.sin(fr), -np.sin(fr)], axis=0)
    return (np.ascontiguousarray(cosf, dtype=BF),
            np.ascontiguousarray(sinf, dtype=BF))


def _masks():
    """0/1 bf16 masks in the TRANSPOSED tile layout [k_loc, q_loc].
    m_low: window lower edge (j=0 tile, qi>=4): allowed iff k > q.
    m_diag: causal/diagonal tile (j=kw-1): allowed iff k <= q."""
    k = np.arange(128)[:, None]
    q = np.arange(128)[None, :]
    m_low = (k > q).astype(BF)
    m_diag = (k <= q).astype(BF)
    return np.ascontiguousarray(m_low), np.ascontiguousarray(m_diag)


def _build_program():
    import concourse.bacc as bacc
    import concourse.tile as tile
    from concourse import mybir

    bf = mybir.dt.bfloat16
    f32 = mybir.dt.float32
    f16 = mybir.dt.float16
    EXP = mybir.ActivationFunctionType.Exp
    MULT = mybir.AluOpType.mult
    ADD = mybir.AluOpType.add
    MAX = mybir.AluOpType.max

    nc = bacc.Bacc("TRN2", target_bir_lowering=False, debug=False,
                   num_devices=N_CORES)

    xt_d = nc.dram_tensor("xt", [128, NKT, S], bf, kind="ExternalInput")
    wq_d = nc.dram_tensor("wq", [128, NKT, 2, 128], bf, kind="ExternalInput")
    wk_d = nc.dram_tensor("wk", [128, NKT, 128], bf, kind="ExternalInput")
    wv_d = nc.dram_tensor("wv", [128, NKT, 128], bf, kind="ExternalInput")
    wo_d = nc.dram_tensor("wo", [128, 2, D], bf, kind="ExternalInput")
    lamn_d = nc.dram_tensor("lamn", [1, 2], f32, kind="ExternalInput")
    out_d = nc.dram_tensor("outp", [S, D], f16, kind="ExternalOutput")

    tabc_np, tabs_np = _tables()
    mlow_np, mdiag_np = _masks()
    tabc_d = nc.inline_tensor(tabc_np, "tab_c")
    tabs_d = nc.inline_tensor(tabs_np, "tab_s")
    mlow_d = nc.inline_tensor(mlow_np, "mask_low")
    mdiag_d = nc.inline_tensor(mdiag_np, "mask_diag")
    idb_d = nc.inline_tensor(np.eye(128, dtype=BF), "ident_bf")
    ones_d = nc.inline_tensor(np.ones((128, 1), dtype=BF), "ones_col")
    onesr_d = nc.inline_tensor(np.ones((1, 128), dtype=BF), "ones_row")

    with tile.TileContext(nc) as tc:
        with tc.tile_pool(name="wp", bufs=1) as wp, \
             tc.tile_pool(name="sb", bufs=1) as sb, \
             tc.tile_pool(name="pbig", bufs=1, space="PSUM") as pbig, \
             tc.tile_pool(name="pst", bufs=1, space="PSUM") as pst, \
             tc.tile_pool(name="psm", bufs=1, space="PSUM") as psm, \
             tc.tile_pool(name="pav", bufs=1, space="PSUM") as pav:

            # ---- weights / tables / constants, ordered first-needed-first ----
            wq = wp.tile([128, NKT, 2, 128], bf)
            nc.sync.dma_start(out=wq[:, 0:8], in_=wq_d[:, 0:8])
            xt = wp.tile([128, NKT, S], bf)
            nc.sync.dma_start(out=xt[:, 0:4, 0:512], in_=xt_d[:, 0:4, 0:512])
            nc.sync.dma_start(out=wq[:, 8:16], in_=wq_d[:, 8:16])
            nc.sync.dma_start(out=xt[:, 4:8, 0:512], in_=xt_d[:, 4:8, 0:512])
            nc.sync.dma_start(out=xt[:, 8:16, 0:512], in_=xt_d[:, 8:16, 0:512])
            wk = wp.tile([128, NKT, 128], bf)
            nc.sync.dma_start(out=wk[:], in_=wk_d[:])
            wv = wp.tile([128, NKT, 128], bf)
            nc.sync.dma_start(out=wv[:], in_=wv_d[:])
            tabc = wp.tile([128, S], bf)
            tabs = wp.tile([128, S], bf)
            nc.sync.dma_start(out=tabc[:], in_=tabc_d[:])
            nc.sync.dma_start(out=tabs[:], in_=tabs_d[:])
            mlow = wp.tile([128, 128], bf)
            nc.sync.dma_start(out=mlow[:], in_=mlow_d[:])
            mdiag = wp.tile([128, 128], bf)
            nc.sync.dma_start(out=mdiag[:], in_=mdiag_d[:])
            idb = wp.tile([128, 128], bf)
            nc.sync.dma_start(out=idb[:], in_=idb_d[:])
            ones1 = wp.tile([128, 1], bf)
            nc.sync.dma_start(out=ones1[:], in_=ones_d[:])
            onesr = wp.tile([1, 128], bf)
            nc.sync.dma_start(out=onesr[:], in_=onesr_d[:])
            lamn = wp.tile([1, 2], f32)
            nc.sync.dma_start(out=lamn[:], in_=lamn_d[:])
            nc.sync.dma_start(out=xt[:, :, 512:1024], in_=xt_d[:, :, 512:1024])
            wo = wp.tile([128, 2, D], bf)
            nc.sync.dma_start(out=wo[:], in_=wo_d[:])
            nc.sync.dma_start(out=xt[:, :, 1024:1536], in_=xt_d[:, :, 1024:1536])
            nc.sync.dma_start(out=xt[:, :, 1536:2048], in_=xt_d[:, :, 1536:2048])

            # ---- persistent activations ----
            # qt_t groups both heads per q-tile so the QK moving operand is a
            # contiguous [64, 256] covering both heads in one matmul
            qt_t = wp.tile([128, NQT, 2, 128], bf)
            kt = wp.tile([128, S], bf)          # RoPE'd k, hd-major
            vsm = wp.tile([128, NQT, 128], bf)  # v, s-major [k_loc, hd]

            # ================= emission helpers =================

            def proj_chain(nch, t):
                """One projection accumulation chain (16 matmuls) for chunk
                nch, target t in {0:q0, 1:q1, 2:k, 3:v}, plus its RoPE or
                v-transpose epilogue."""
                sl = slice(nch * 512, (nch + 1) * 512)
                ps = pbig.tile([128, 512], f32, tag="big", bufs=2)
                if t == 0:
                    w_ap = wq[:, :, 0, :]
                elif t == 1:
                    w_ap = wq[:, :, 1, :]
                elif t == 2:
                    w_ap = wk
                else:
                    w_ap = wv
                for kti in range(NKT):
                    nc.tensor.matmul(ps[:], w_ap[:, kti, :], xt[:, kti, sl],
                                     start=(kti == 0), stop=(kti == NKT - 1))
                f = sb.tile([128, 512], bf, tag="ropef", bufs=3)
                nc.scalar.copy(out=f[:], in_=ps[:])
                if t < 3:
                    outt = (qt_t[:, 4 * nch:4 * nch + 4, t, :]
                            if t < 2 else kt[:, sl])
                    m1 = sb.tile([128, 512], bf, tag="m1", bufs=2)
                    m2 = sb.tile([128, 512], bf, tag="m2", bufs=2)
                    nc.vector.tensor_mul(m1[:], f[:], tabc[:, sl])
                    nc.vector.tensor_mul(m2[0:64, :], f[64:128, :], tabs[64:128, sl])
                    nc.vector.tensor_mul(m2[64:128, :], f[0:64, :], tabs[0:64, sl])
                    nc.vector.tensor_add(outt, m1[:], m2[:])
                else:
                    ptv = pbig.tile([128, 4, 128], bf, tag="big", bufs=2)
                    for j in range(4):
                        nc.tensor.transpose(ptv[:, j, :],
                                            f[:, 128 * j:128 * (j + 1)], idb[:])
                    nc.vector.tensor_copy(out=vsm[:, 4 * nch:4 * (nch + 1), :],
                                          in_=ptv[:])

            # attention pipeline state carried between stages
            st = {}

            def kw_of(qi):
                return min(qi + 1, 5)

            def s1_emit(qi):
                """Stage 1 per q-tile, BOTH heads: transposed QK (N=256 over
                both heads) + exp + edge masks. e_all layout [128, j, h,
                half, q] keeps the per-(j,h) sum operand contiguous."""
                kw = kw_of(qi)
                g0 = max(0, qi - 4)
                e_all = sb.tile([128, 5, 2, 2, 128], bf, tag="eall", bufs=3)
                joff = 0
                while joff < kw:
                    jw = min(2, kw - joff)
                    sT = pst.tile([128, 2, 2, 2, 128], f32, tag="sT", bufs=2)
                    for j in range(jw):
                        g = g0 + joff + j
                        ksl = slice(g * 128, (g + 1) * 128)
                        nc.tensor.matmul(sT[:, 0, j, :, :], kt[0:64, ksl],
                                         qt_t[0:64, qi, :, :],
                                         start=True, stop=True)
                        nc.tensor.matmul(sT[:, 1, j, :, :], kt[64:128, ksl],
                                         qt_t[64:128, qi, :, :],
                                         start=True, stop=True)
                    nc.scalar.activation(
                        out=e_all[:, joff:joff + jw, :, :, :]
                            .transpose([0, 3, 1, 2, 4]),
                        in_=sT[:, :, 0:jw, :, :], func=EXP)
                    joff += jw
                # masks: j=0 lower-window edge (only when qi>=4), j=kw-1 diagonal
                if qi >= 4:
                    nc.vector.tensor_mul(
                        e_all[:, 0, :, :, :], e_all[:, 0, :, :, :],
                        mlow[:, None, None, :].broadcast_to([128, 2, 2, 128]))
                nc.vector.tensor_mul(
                    e_all[:, kw - 1, :, :, :], e_all[:, kw - 1, :, :, :],
                    mdiag[:, None, None, :].broadcast_to([128, 2, 2, 128]))
                st[("e", qi)] = e_all

            def s2a_emit(i):
                """Stage 2a: ones-matmul sums of e1/e2, cneg scalar chain
                (with the per-partition broadcast done by a K=1 matmul), and
                the gt = relu(e1 + cneg*e2) combine."""
                qi, h = i // 2, i % 2
                kw = kw_of(qi)
                e_all = st.pop(("e", qi)) if h == 1 else st[("e", qi)]
                # sm2 bank layout: [0:1, 0:256] = s1|s2 sums,
                #                  [:, 256:384] = cnegB broadcast
                sm = psm.tile([128, 384], f32, tag="sm", bufs=1)
                for j in range(kw):
                    nc.tensor.matmul(sm[0:1, 0:256],
                                     ones1[:], e_all[:, j, h, :, :],
                                     start=(j == 0), stop=(j == kw - 1))
                r2 = sb.tile([1, 128], f32, tag="r2", bufs=3)
                nc.vector.reciprocal_approx_fast(out=r2[:], in_=sm[0:1, 128:256])
                cneg = sb.tile([1, 128], bf, tag="cneg", bufs=3)
                nc.vector.scalar_tensor_tensor(
                    out=cneg[:], in0=sm[0:1, 0:128], scalar=lamn[0:1, h:h + 1],
                    in1=r2[:], op0=MULT, op1=MULT)
                nc.tensor.matmul(sm[:, 256:384], onesr[:], cneg[:],
                                 start=True, stop=True)
                # gt = relu(e1 + cneg * e2)   (vector, k-major)
                gt = sb.tile([128, 5, 128], bf, tag="gt", bufs=3)
                nc.vector.tensor_mul(
                    gt[:, 0:kw, :], e_all[:, 0:kw, h, 1, :],
                    sm[:, None, 256:384].broadcast_to([128, kw, 128]))
                nc.vector.tensor_add(gt[:, 0:kw, :], gt[:, 0:kw, :],
                                     e_all[:, 0:kw, h, 0, :])
                nc.vector.tensor_scalar(out=gt[:, 0:kw, :], in0=gt[:, 0:kw, :],
                                        scalar1=0.0, scalar2=0.0,
                                        op0=MAX, op1=ADD)
                st[("gt", i)] = gt

            def s2b_emit(i):
                """Stage 2b: dsum chain, AV chain, normalize into att."""
                qi, h = i // 2, i % 2
                kw = kw_of(qi)
                g0 = max(0, qi - 4)
                gt = st.pop(("gt", i))
                if h == 0:
                    av = pav.tile([128, 4, 128], f32, tag="av", bufs=1)
                    att = sb.tile([128, 2, 128], bf, tag="att", bufs=2)
                    st[("av", qi)] = av
                    st[("att", qi)] = att
                else:
                    av = st[("av", qi)]
                    att = st[("att", qi)]
                # dsum into av tile partition-0 row region [1,128] at slot 2+h
                nc.tensor.matmul(av[0:1, 2 + h, :], ones1[:], gt[:, 0, :],
                                 start=True, stop=(kw == 1))
                for j in range(1, kw):
                    nc.tensor.matmul(av[0:1, 2 + h, :], ones1[:], gt[:, j, :],
                                     start=False, stop=(j == kw - 1))
                for j in range(kw):
                    nc.tensor.matmul(av[:, h, :], vsm[:, g0 + j, :], gt[:, j, :],
                                     start=(j == 0), stop=(j == kw - 1))
                recd = sb.tile([1, 128], f32, tag="recd", bufs=2)
                nc.vector.reciprocal_approx_fast(out=recd[:], in_=av[0:1, 2 + h, :])
                recdB = sb.tile([128, 128], f32, tag="recdB", bufs=2)
                nc.gpsimd.partition_broadcast(recdB[:], recd[:])
                nc.vector.tensor_mul(att[:, h, :], av[:, h, :], recdB[:])

            def p3_emit(qi):
                """Output projection for one q-tile (row-sharded Wo partial)."""
                qsl = slice(qi * 128, (qi + 1) * 128)
                att = st.pop(("att", qi))
                st.pop(("av", qi))
                for dch in range(4):
                    dsl = slice(dch * 512, (dch + 1) * 512)
                    ps_o = pbig.tile([128, 512], f32, tag="big", bufs=2)
                    nc.tensor.matmul(ps_o[:], att[:, 0, :], wo[:, 0, dsl],
                                     start=True, stop=False)
                    nc.tensor.matmul(ps_o[:], att[:, 1, :], wo[:, 1, dsl],
                                     start=False, stop=True)
                    so = sb.tile([128, 512], f16, tag="so", bufs=6)
                    nc.any.tensor_copy(out=so[:], in_=ps_o[:])
                    nc.sync.dma_start(out=out_d[qsl, dsl], in_=so[:])

            # ================= emission schedule =================
            for t in range(4):
                proj_chain(0, t)
            for t in range(4):
                proj_chain(1, t)
            proj_left = [(2, 0), (2, 1), (2, 2), (2, 3),
                         (3, 0), (3, 1), (3, 2), (3, 3)]
            NI = 32
            for i in range(NI + 3):
                if i < NI and i % 2 == 0:
                    s1_emit(i // 2)
                if 0 <= i - 1 < NI:
                    s2a_emit(i - 1)
                if 0 <= i - 3 < NI:
                    s2b_emit(i - 3)
                    if (i - 3) % 2 == 1:
                        p3_emit((i - 3) // 2)
                if proj_left and i % 2 == 1:
                    nch, t = proj_left.pop(0)
                    proj_chain(nch, t)

    nc.compile()
    return nc


def get_program():
    if "nc" not in _CACHE:
        _CACHE["nc"] = _build_program()
    return _CACHE["nc"]


def _prep_inputs(x, Wq, Wk, Wv, Wo, lam):
    xt = np.ascontiguousarray(x.reshape(S, D).T.astype(BF)
                              .reshape(NKT, 128, S).transpose(1, 0, 2))
    in_maps = []
    for c in range(N_CORES):
        h0 = 2 * c
        kv = c // 2
        # attention scale 1/sqrt(64) folded into Wq (exact pow2 in bf16)
        wq_c = np.ascontiguousarray(
            (Wq[:, h0 * 128:(h0 + 2) * 128] * 0.125).astype(BF)
            .reshape(NKT, 128, 2, 128).transpose(1, 0, 2, 3))
        wk_c = np.ascontiguousarray(
            Wk[:, kv * 128:(kv + 1) * 128].astype(BF)
            .reshape(NKT, 128, 128).transpose(1, 0, 2))
        wv_c = np.ascontiguousarray(
            Wv[:, kv * 128:(kv + 1) * 128].astype(BF)
            .reshape(NKT, 128, 128).transpose(1, 0, 2))
        wo_c = np.ascontiguousarray(
            Wo[h0 * 128:(h0 + 2) * 128, :].astype(BF)
            .reshape(2, 128, D).transpose(1, 0, 2))
        lamn_c = np.array([[-float(lam[h0]), -float(lam[h0 + 1])]], dtype=np.float32)
        in_maps.append({"xt": xt, "wq": wq_c, "wk": wk_c, "wv": wv_c,
                        "wo": wo_c, "lamn": lamn_c})
    return in_maps


def kernel(x, Wq, Wk, Wv, Wo, lam):
    from concourse.bass_utils import run_bass_kernel_spmd

    nc = get_program()
    in_maps = _prep_inputs(np.asarray(x), np.asarray(Wq), np.asarray(Wk),
                           np.asarray(Wv), np.asarray(Wo), np.asarray(lam))
    res = run_bass_kernel_spmd(nc, in_maps, list(range(N_CORES)))
    out = np.zeros((S, D), dtype=np.float32)
    for c in range(N_CORES):
        out += res.results[c]["outp"].astype(np.float32)
    return out.reshape(1, S, D)
